# revision 24
# baseline (speedup 1.0000x reference)
"""CapsuleNet Trainium2 kernel: 8-core data-parallel (batch 256 -> 32/core).

Self-contained: hardcodes all shapes from the problem spec.
Returns (classes [256,10], recon [256,784]) like the reference.
"""
import numpy as np

import concourse.bass as bass
import concourse.mybir as mybir
import concourse.tile as tile
from concourse import bacc
from concourse import bass_utils

N_CORES = 8
BC = 32          # batch per core
F32 = mybir.dt.float32
F32R = mybir.dt.float32r
AF = mybir.ActivationFunctionType
OP = mybir.AluOpType
AX = mybir.AxisListType


def fp32r_round(x):
    x = np.ascontiguousarray(x, np.float32)
    u = x.view(np.uint32)
    r = ((u.astype(np.uint64) + 0x800) & 0xFFFFF000).astype(np.uint32)
    return r.view(np.float32)


def _ap(t_ap, off, dims):
    """Build an AP on the same tensor with partition dim kept, custom free dims."""
    return bass.AP(tensor=t_ap.tensor, offset=t_ap.offset + off,
                   ap=[list(t_ap.ap[0])] + [list(d) for d in dims])


import os
STAGE = int(os.environ.get("KSTAGE", "9"))


def build_program():
    nc = bacc.Bacc("TRN2", target_bir_lowering=False, debug=False,
                   num_devices=N_CORES)

    # ---------------- DRAM I/O ----------------
    d_x = nc.dram_tensor("x", [BC, 784], F32R, kind="ExternalInput").ap()
    d_w0 = nc.dram_tensor("w0", [81, 256], F32R, kind="ExternalInput").ap()
    d_b0 = nc.dram_tensor("b0", [256], F32, kind="ExternalInput").ap()
    d_w1 = nc.dram_tensor("w1", [2, 81, 2, 128, 128], F32R, kind="ExternalInput").ap()
    d_b1 = nc.dram_tensor("b1", [256], F32, kind="ExternalInput").ap()
    d_cfc = nc.dram_tensor("cfc", [257, 288], F32, kind="ExternalInput").ap()
    d_route = nc.dram_tensor("route", [36, 8, 160], F32, kind="ExternalInput").ap()
    d_dw1 = nc.dram_tensor("dw1", [161, 512], F32R, kind="ExternalInput").ap()
    d_dw2 = nc.dram_tensor("dw2", [513, 1024], F32R, kind="ExternalInput").ap()
    d_dw3 = nc.dram_tensor("dw3", [1025, 784], F32R, kind="ExternalInput").ap()
    d_cls = nc.dram_tensor("classes", [BC, 10], F32, kind="ExternalOutput").ap()
    d_rec = nc.dram_tensor("recon", [BC, 784], F32, kind="ExternalOutput").ap()

    with tile.TileContext(nc) as tc:
        _body(nc, tc, d_x, d_w0, d_b0, d_w1, d_b1, d_cfc, d_route,
              d_dw1, d_dw2, d_dw3, d_cls, d_rec)
    nc.compile()
    return nc


def _body(nc, tc, d_x, d_w0, d_b0, d_w1, d_b1, d_cfc, d_route,
          d_dw1, d_dw2, d_dw3, d_cls, d_rec):
    from contextlib import ExitStack
    ctx = ExitStack()
    persist = ctx.enter_context(tc.tile_pool(name="persist", bufs=1))
    w1p = ctx.enter_context(tc.tile_pool(name="w1p", bufs=6))
    h0p = tc.alloc_tile_pool(name="h0p", bufs=1)
    imcp = tc.alloc_tile_pool(name="imc", bufs=1)
    c1ps = tc.alloc_tile_pool(name="c1ps", bufs=1, space="PSUM")
    c0ps = tc.alloc_tile_pool(name="c0ps", bufs=2, space="PSUM")

    # ---------------- stage A: input loads ----------------
    x_s = persist.tile([BC, 784], F32R, tag="x")
    nc.sync.dma_start(x_s[:], d_x)
    w0_s = persist.tile([81, 256], F32R, tag="w0")
    nc.scalar.dma_start(w0_s[:], d_w0)
    b0_s = persist.tile([128, 2], F32, tag="b0")   # [:,icc]
    b1_s = persist.tile([128, 2], F32, tag="b1")
    for icc in range(2):
        nc.scalar.dma_start(b0_s[:, icc:icc + 1],
                            d_b0[icc * 128:(icc + 1) * 128].unsqueeze(1))
        nc.scalar.dma_start(b1_s[:, icc:icc + 1],
                            d_b1[icc * 128:(icc + 1) * 128].unsqueeze(1))
    cfc0_s = persist.tile([128, 288], F32, tag="cfc0")
    nc.scalar.dma_start(cfc0_s[:], d_cfc[0:128, :])
    cfc1_s = persist.tile([128, 288], F32, tag="cfc1")
    nc.scalar.dma_start(cfc1_s[:], d_cfc[128:256, :])
    cfcb_s = persist.tile([1, 288], F32, tag="cfcb")
    nc.scalar.dma_start(cfcb_s[:], d_cfc[256:257, :])
    route_s = persist.tile([8, 36, 160], F32, tag="route")
    nc.scalar.dma_start(route_s[:], d_route.rearrange("p d co -> d p co"))
    ones_s = persist.tile([1, 32], F32, tag="ones")
    nc.vector.memset(ones_s[:], 1.0)

    # ---------------- stage A2: im2col gather (SBUF->SBUF) ----------------
    # imc[bc] [81, 8*400] f32r ; row (ky,kx) col (b,oy,ox) = x[b, oy+ky, ox+kx]
    imcs = []
    for bc in range(4):
        imt = imcp.tile([81, 8 * 400], F32R, tag=f"imc{bc}")
        imcs.append(imt)
        for ky in range(9):
            for kx in range(9):
                src = _ap(x_s[bc * 8:(bc + 1) * 8, :], ky * 28 + kx,
                          [[28, 20], [1, 20]])
                eng = (nc.sync, nc.scalar)[bc % 2]
                eng.dma_start(imt[ky * 9 + kx:ky * 9 + kx + 1, :].rearrange(
                    "p (b f) -> p b f", b=8), src)

    # ---------------- stage B: conv0 ----------------
    # h0 tiles [128, 8, 20, 10] per (icc, ph, bc), f32r, value relu(conv0+b0)
    h0 = [[[None] * 4 for _ in range(2)] for _ in range(2)]
    for icc in range(2):
        for bc in range(4):
            h0[icc][0][bc] = h0p.tile([128, 8, 20, 10], F32R, tag=f"h0_{icc}_0_{bc}", name=f"h0_{icc}_0_{bc}")
            h0[icc][1][bc] = h0p.tile([128, 8, 20, 10], F32R, tag=f"h0_{icc}_1_{bc}", name=f"h0_{icc}_1_{bc}")
    for icc in range(2):
        lhs = w0_s[:, icc * 128:(icc + 1) * 128]
        for bc in range(4):
            for pair in range(4):   # 2 samples per psum tile
                ps = c0ps.tile([128, 1024], F32, tag="c0")
                for s in range(2):
                    b_loc = pair * 2 + s
                    nc.tensor.matmul(
                        ps[:, s * 512:s * 512 + 400], lhs,
                        imcs[bc][:, b_loc * 400:(b_loc + 1) * 400].rearrange(
                            "p (o f) -> p o f", o=1),
                        start=True, stop=True)
                # phase-split relu copies: ACT does ph0, DVE does ph1
                for ph in range(2):
                    src = _ap(ps[:], ph, [[512, 2], [20, 20], [2, 10]])
                    dst = h0[icc][ph][bc][:, pair * 2:pair * 2 + 2, :, :]
                    if ph == 0:
                        nc.scalar.activation(out=dst, in_=src, func=AF.Relu,
                                             bias=b0_s[:, icc:icc + 1], scale=1.0)
                    else:
                        nc.vector.tensor_scalar(
                            out=dst, in0=src, scalar1=b0_s[:, icc:icc + 1],
                            scalar2=0.0, op0=OP.add, op1=OP.max)

    if STAGE <= 1:
        # dump h0[0][0][0] [128, 1600] -> rec[0:32, :784] and quit
        nc.sync.dma_start(d_rec,
                          h0[0][0][0][:].rearrange("q a b c -> q (a b c)")
                          [0:32, 0:784].bitcast(F32))
        nc.vector.memset(cls_z := persist.tile([BC, 10], F32, tag="clsz", name="clsz"), 0.0)
        nc.sync.dma_start(d_cls, cls_z[:])
        c0ps.release(); c1ps.release(); imcp.release(); h0p.release()
        ctx.close()
        return

    # ---------------- stage C: conv1 ----------------
    imcp.release()
    c0ps.release()
    # psum [128oc, 288] per bc; oc-half sequential; 2x162 weight DMAs (64KB)
    h1 = [persist.tile([128, BC * 36], F32, tag=f"h1_{oh}", name=f"h1_{oh}") for oh in range(2)]
    C1N = int(os.environ.get("KC1N", "162"))
    C1OH = int(os.environ.get("KC1OH", "2"))
    C1RES = os.environ.get("KC1RES", "0") == "1"
    wres = None
    if C1RES:
        wres = [w1p.tile([128, 128], F32R, tag=f"wres{j}", name=f"wres{j}", bufs=1)
                for j in range(18)]
        for j in range(18):
            nc.sync.dma_start(wres[j][:], d_w1[0, j, 0, :, :])
    for oh in range(C1OH):
        c1p = [c1ps.tile([128, 288], F32, tag=f"c1_{bc}", name=f"c1_{oh}_{bc}")
               for bc in range(4)]
        for i in range(C1N):
            kk, icc = i // 2, i % 2
            ky, kx = kk // 9, kk % 9
            if C1RES:
                wt = wres[i % 18]
            else:
                wt = w1p.tile([128, 128], F32R, tag="w1t")
                nc.sync.dma_start(wt[:], d_w1[oh, kk, icc, :, :])
            ph, kxh = kx % 2, kx // 2
            for bc in range(4):
                rhs = _ap(h0[icc][ph][bc][:], ky * 10 + kxh,
                          [[200, 8], [20, 6], [1, 6]])
                nc.tensor.matmul(c1p[bc][:], wt[:], rhs,
                                 start=(i == 0), stop=(i == C1N - 1))
        for bc in range(4):
            nc.scalar.activation(
                out=h1[oh][:, bc * 8 * 36:(bc + 1) * 8 * 36],
                in_=c1p[bc][:], func=AF.Relu,
                bias=b1_s[:, oh:oh + 1], scale=1.0)

    if STAGE <= 2:
        nc.sync.dma_start(d_rec, h1[0][0:32, 0:784])
        nc.vector.memset(cls_z := persist.tile([BC, 10], F32, tag="clsz", name="clsz"), 0.0)
        nc.sync.dma_start(d_cls, cls_z[:])
        c1ps.release(); h0p.release()
        ctx.close()
        return

    # ---------------- stage E: CFC -> u [32, (36,8)] ----------------
    h0p.release()
    c1ps.release()
    tailps = ctx.enter_context(tc.tile_pool(name="tailps", bufs=1, space="PSUM"))
    pu = tailps.tile([BC, 288], F32, tag="pu")
    h1v = [h1[oh][:].rearrange("q (b p) -> q b p", p=36) for oh in range(2)]
    for p in range(36):
        for icc in range(2):
            nc.tensor.matmul(pu[:, p * 8:(p + 1) * 8], h1v[icc][:, :, p],
                             cfc0_s[:, p * 8:(p + 1) * 8] if icc == 0
                             else cfc1_s[:, p * 8:(p + 1) * 8],
                             start=(icc == 0), stop=False)
        nc.tensor.matmul(pu[:, p * 8:(p + 1) * 8], ones_s[:],
                         cfcb_s[:, p * 8:(p + 1) * 8], start=False, stop=True)

    tail = ctx.enter_context(tc.tile_pool(name="tail", bufs=1))
    u_s = tail.tile([BC, 320], F32, tag="u")
    nc.vector.tensor_copy(out=u_s[:, 0:288], in_=pu[:])
    nc.vector.memset(u_s[:, 288:320], 0.0)

    # squash u along d=8
    u2 = tail.tile([BC, 288], F32, tag="u2")
    nc.vector.tensor_mul(u2[:], u_s[:, 0:288], u_s[:, 0:288])
    sq = tail.tile([BC, 36], F32, tag="sq")
    nc.vector.tensor_reduce(sq[:], u2[:].rearrange("b (p d) -> b p d", d=8),
                            axis=AX.X, op=OP.add)
    sf = _squash_factor(nc, tail, sq, 36, "uf")          # [32, 36]
    usq = tail.tile([BC, 320], F32, tag="usq")
    nc.vector.tensor_tensor(
        out=usq[:, 0:288].rearrange("b (p d) -> b p d", d=8),
        in0=u_s[:, 0:288].rearrange("b (p d) -> b p d", d=8),
        in1=_ap(sf[:], 0, [[1, 36], [0, 8]]), op=OP.mult)
    nc.vector.memset(usq[:, 288:320], 0.0)

    if STAGE <= 3:
        nc.vector.memset(pad_z := tail.tile([BC, 784], F32, tag="padz", name="padz"), 0.0)
        nc.vector.tensor_copy(out=pad_z[:, 0:288], in_=usq[:, 0:288])
        nc.sync.dma_start(d_rec, pad_z[:])
        nc.vector.memset(cls_z := tail.tile([BC, 10], F32, tag="clsz", name="clsz"), 0.0)
        nc.sync.dma_start(d_cls, cls_z[:])
        ctx.close()
        return

    # ---------------- stage G: priors [32b, (p,c,o)] ----------------
    pri = tail.tile([BC, 36 * 160], F32, tag="pri")
    utp = ctx.enter_context(tc.tile_pool(name="utp", bufs=3))
    prps = ctx.enter_context(tc.tile_pool(name="prps", bufs=3, space="PSUM"))
    for p in range(36):
        ut = utp.tile([32, 32], F32, tag="ut")
        nc.vector.transpose(ut[:], usq[:, p * 8:p * 8 + 32])
        pp = prps.tile([BC, 160], F32, tag="pp")
        nc.tensor.matmul(pp[:], ut[0:8, :], route_s[:, p, :],
                         start=True, stop=True)
        nc.scalar.copy(pri[:, p * 160:(p + 1) * 160], pp[:])

    # ---------------- stage H: re-layout to (c,b) chunks ----------------
    # chunks: pc[k] [(4c x 32b) or (2c x 32b), (p36, o16)]
    nch = [4, 4, 2]
    pc = [tail.tile([nch[k] * 32, 576], F32, tag=f"pc{k}", name=f"pc{k}") for k in range(3)]
    for c in range(10):
        k, cl = c // 4, c % 4
        src = _ap(pri[:], c * 16, [[160, 36], [1, 16]])
        eng = (nc.sync, nc.scalar)[c % 2]
        eng.dma_start(pc[k][cl * 32:(cl + 1) * 32, :].rearrange(
            "q (p o) -> q p o", o=16), src)

    if STAGE <= 4:
        nc.sync.dma_start(
            bass.AP(tensor=d_rec.tensor, offset=d_rec.offset,
                    ap=[[784, 32], [1, 576]]), pc[0][0:32, 0:576])
        nc.vector.memset(pad_q := tail.tile([BC, 208], F32, tag="padq", name="padq"), 0.0)
        nc.sync.dma_start(
            bass.AP(tensor=d_rec.tensor, offset=d_rec.offset + 576,
                    ap=[[784, 32], [1, 208]]), pad_q[:])
        nc.vector.memset(cls_z := tail.tile([BC, 10], F32, tag="clsz", name="clsz"), 0.0)
        nc.sync.dma_start(d_cls, cls_z[:])
        ctx.close()
        return

    # ---------------- stage I: routing (3 iters) ----------------
    outs, nrms = [], []
    for k in range(3):
        P = nch[k] * 32
        t = pc[k]
        po = t[:].rearrange("q (p o) -> q p o", o=16)      # [P, p, o]
        op_v = _ap(t[:], 0, [[1, 16], [16, 36]])           # [P, o, p]
        tmp = tail.tile([P, 576], F32, tag=f"tmp{k}")
        tmp_po = tmp[:].rearrange("q (p o) -> q p o", o=16)
        # tmp2: o-major contiguous scratch [P, (o,p)]
        tmp2 = tail.tile([P, 576], F32, tag=f"tmp2_{k}")
        tmp2_op = tmp2[:].rearrange("q (o p) -> q o p", p=36)
        lg = tail.tile([P, 36], F32, tag=f"lg{k}")
        ex = tail.tile([P, 36], F32, tag=f"ex{k}")
        orw = tail.tile([P, 16], F32, tag=f"orw{k}")
        out = tail.tile([P, 16], F32, tag=f"out{k}")
        sqr = tail.tile([P, 1], F32, tag=f"sqr{k}")
        red = tail.tile([P, 36], F32, tag=f"red{k}")

        def soft_out(probs_src, scale, it):
            # o_raw = sum_p probs*priors  (probs_src=None -> plain sum)
            if probs_src is None:
                nc.vector.tensor_copy(out=tmp2_op, in_=op_v)
            else:
                nc.vector.tensor_tensor(
                    out=tmp2_op, in0=op_v,
                    in1=_ap(probs_src[:], 0, [[0, 16], [1, 36]]), op=OP.mult)
            nc.vector.tensor_reduce(orw[:], tmp2_op, axis=AX.X, op=OP.add)
            # squash(o_raw*scale) = o_raw * g
            nc.vector.tensor_mul(tmp[:, 0:16], orw[:], orw[:])
            nc.vector.tensor_reduce(sqr[:], tmp[:, 0:16], axis=AX.X, op=OP.add)
            g, s_t = _squash_scaled(nc, tail, sqr, scale, f"g{k}_{it}")
            nc.vector.tensor_scalar(out=out[:], in0=orw[:], scalar1=g[:],
                                    scalar2=None, op0=OP.mult)
            return g, s_t

        def softmax(src):
            nc.scalar.activation(out=ex[:], in_=src[:], func=AF.Exp,
                                 bias=0.0, scale=1.0)
            sm = tail.tile([P, 1], F32, tag=f"sm{k}")
            nc.vector.tensor_reduce(sm[:], ex[:], axis=AX.X, op=OP.add)
            nc.vector.reciprocal(sm[:], sm[:])
            pr = tail.tile([P, 36], F32, tag=f"pr{k}")
            nc.vector.tensor_scalar(out=pr[:], in0=ex[:], scalar1=sm[:],
                                    scalar2=None, op0=OP.mult)
            return pr

        KSUB = int(os.environ.get("KSUB", "9"))
        # iter 0: probs uniform = 1/36
        g, s_t = soft_out(None, 1.0 / 36.0, 0)
        if KSUB >= 2:
            # logits1 = sum_o priors*out0
            nc.vector.tensor_tensor(out=tmp_po, in0=po,
                                    in1=_ap(out[:], 0, [[0, 36], [1, 16]]),
                                    op=OP.mult)
            nc.vector.tensor_reduce(lg[:], tmp_po, axis=AX.X, op=OP.add)
        if KSUB >= 3:
            # iter 1
            pr = softmax(lg)
            g, s_t = soft_out(pr, 1.0, 1)
        if KSUB >= 4:
            nc.vector.tensor_tensor(out=tmp_po, in0=po,
                                    in1=_ap(out[:], 0, [[0, 36], [1, 16]]),
                                    op=OP.mult)
            nc.vector.tensor_reduce(red[:], tmp_po, axis=AX.X, op=OP.add)
            nc.vector.tensor_add(lg[:], lg[:], red[:])
            # iter 2
            pr = softmax(lg)
            g, s_t = soft_out(pr, 1.0, 2)
        # final norm = g * s, replicated to 4 cols for 16B-wide DMA gather
        nr4 = tail.tile([P, 4], F32, tag=f"nr4_{k}")
        for j in range(4):
            nc.vector.tensor_mul(nr4[:, j:j + 1], g[:], s_t[:])
        outs.append(out)
        nrms.append(nr4)

    if STAGE <= 5:
        nc.vector.memset(pad_z := tail.tile([BC, 784], F32, tag="padz", name="padz"), 0.0)
        for c in range(10):
            k, cl = c // 4, c % 4
            nc.sync.dma_start(pad_z[:, c * 16:(c + 1) * 16],
                              outs[k][cl * 32:(cl + 1) * 32, :])
        nc.sync.dma_start(d_rec, pad_z[:])
        nb4_t = tail.tile([BC, 40], F32, tag="nb4t")
        if os.environ.get("KNB", "1") == "1":
            for c in range(10):
                k, cl = c // 4, c % 4
                nc.sync.dma_start(nb4_t[:, c * 4:(c + 1) * 4],
                                  nrms[k][cl * 32:(cl + 1) * 32, :])
        else:
            nc.vector.memset(nb4_t[:], 0.0)
        nb_t = tail.tile([BC, 10], F32, tag="nbt")
        nc.vector.tensor_copy(out=nb_t[:], in_=_ap(nb4_t[:], 0, [[4, 10]]))
        nc.sync.dma_start(d_cls, nb_t[:])
        ctx.close()
        return

    # ---------------- stage J: classes + argmax mask ----------------
    nb4 = tail.tile([BC, 40], F32, tag="nb4")
    for c in range(10):
        k, cl = c // 4, c % 4
        nc.sync.dma_start(nb4[:, c * 4:(c + 1) * 4],
                          nrms[k][cl * 32:(cl + 1) * 32, :])
    nb = tail.tile([BC, 10], F32, tag="nb")
    nc.vector.tensor_copy(out=nb[:], in_=_ap(nb4[:], 0, [[4, 10]]))
    rmax = tail.tile([BC, 1], F32, tag="rmax")
    nc.vector.tensor_reduce(rmax[:], nb[:], axis=AX.X, op=OP.max)
    exn = tail.tile([BC, 10], F32, tag="exn")
    nc.scalar.activation(out=exn[:], in_=nb[:], func=AF.Exp, bias=0.0, scale=1.0)
    sme = tail.tile([BC, 1], F32, tag="sme")
    nc.vector.tensor_reduce(sme[:], exn[:], axis=AX.X, op=OP.add)
    nc.vector.reciprocal(sme[:], sme[:])
    cls = tail.tile([BC, 10], F32, tag="cls")
    nc.vector.tensor_scalar(out=cls[:], in0=exn[:], scalar1=sme[:],
                            scalar2=None, op0=OP.mult)
    nc.sync.dma_start(d_cls, cls[:])
    yb4 = tail.tile([BC, 40], F32, tag="yb4")
    nc.vector.tensor_scalar(out=yb4[:], in0=nb4[:], scalar1=rmax[:],
                            scalar2=None, op0=OP.is_ge)
    ycb = [tail.tile([nch[k] * 32, 4], F32, tag=f"ycb{k}", name=f"ycb{k}") for k in range(3)]
    for c in range(10):
        k, cl = c // 4, c % 4
        nc.scalar.dma_start(ycb[k][cl * 32:(cl + 1) * 32, :],
                            yb4[:, c * 4:(c + 1) * 4])

    # ---------------- stage K: mask + decoder ----------------
    mk = [tail.tile([nch[k] * 32, 32], F32, tag=f"mk{k}", name=f"mk{k}") for k in range(3)]
    for k in range(3):
        nc.vector.tensor_scalar(out=mk[k][:, 0:16], in0=outs[k][:],
                                scalar1=ycb[k][:, 0:1], scalar2=None, op0=OP.mult)
        nc.vector.memset(mk[k][:, 16:32], 0.0)

    decp = ctx.enter_context(tc.tile_pool(name="decp", bufs=1))
    decps = ctx.enter_context(tc.tile_pool(name="decps", bufs=1, space="PSUM"))
    # decoder weights
    wa1 = [decp.tile([16, 512], F32R, tag=f"wa1_{c}", name=f"wa1_{c}") for c in range(10)]
    for c in range(10):
        nc.sync.dma_start(wa1[c][:], d_dw1[c * 16:(c + 1) * 16, :])
    wb1 = decp.tile([1, 512], F32R, tag="wb1")
    nc.sync.dma_start(wb1[:], d_dw1[160:161, :])
    wa2 = [decp.tile([128, 1024], F32R, tag=f"wa2_{j}", name=f"wa2_{j}") for j in range(4)]
    for j in range(4):
        nc.scalar.dma_start(wa2[j][:], d_dw2[j * 128:(j + 1) * 128, :])
    wb2 = decp.tile([1, 1024], F32R, tag="wb2")
    nc.scalar.dma_start(wb2[:], d_dw2[512:513, :])
    wa3 = [decp.tile([128, 784], F32R, tag=f"wa3_{j}", name=f"wa3_{j}") for j in range(8)]
    for j in range(8):
        nc.sync.dma_start(wa3[j][:], d_dw3[j * 128:(j + 1) * 128, :])
    wb3 = decp.tile([1, 784], F32R, tag="wb3")
    nc.sync.dma_start(wb3[:], d_dw3[1024:1025, :])
    onesr = decp.tile([1, 32], F32R, tag="onesr")
    nc.vector.tensor_copy(out=onesr[:], in_=ones_s[:])

    # fc1: maskedT chunks via DVE transpose (f32) + cast to f32r
    mtf = [decp.tile([32, 32], F32, tag=f"mtf{c}", name=f"mtf{c}") for c in range(10)]
    mtp = [decp.tile([32, 32], F32R, tag=f"mtp{c}", name=f"mtp{c}") for c in range(10)]
    for c in range(10):
        k, cl = c // 4, c % 4
        nc.vector.transpose(mtf[c][:], mk[k][cl * 32:(cl + 1) * 32, :])
        nc.vector.tensor_copy(out=mtp[c][:], in_=mtf[c][:])
    pf1 = decps.tile([BC, 512], F32, tag="pf1")
    for c in range(10):
        nc.tensor.matmul(pf1[:], mtp[c][0:16, :], wa1[c][:],
                         start=(c == 0), stop=False)
    nc.tensor.matmul(pf1[:], onesr[:], wb1[:], start=False, stop=True)
    h1d = decp.tile([BC, 512], F32, tag="h1d")
    nc.scalar.activation(out=h1d[:], in_=pf1[:], func=AF.Relu, bias=0.0, scale=1.0)

    ht1f = [decp.tile([128, 32], F32, tag=f"ht1f_{j}", name=f"ht1f_{j}") for j in range(4)]
    ht1 = [decp.tile([128, 32], F32R, tag=f"ht1_{j}", name=f"ht1_{j}") for j in range(4)]
    for j in range(16):
        nc.vector.transpose(ht1f[j // 4][(j % 4) * 32:(j % 4 + 1) * 32, :],
                            h1d[:, j * 32:(j + 1) * 32])
    for j in range(4):
        nc.vector.tensor_copy(out=ht1[j][:], in_=ht1f[j][:])
    h2d = decp.tile([BC, 1024], F32, tag="h2d")
    for half in range(2):
        pf2 = decps.tile([BC, 512], F32, tag="pf2")
        for j in range(4):
            nc.tensor.matmul(pf2[:], ht1[j][:],
                             wa2[j][:, half * 512:(half + 1) * 512],
                             start=(j == 0), stop=False)
        nc.tensor.matmul(pf2[:], onesr[:],
                         wb2[:, half * 512:(half + 1) * 512],
                         start=False, stop=True)
        nc.scalar.activation(out=h2d[:, half * 512:(half + 1) * 512],
                             in_=pf2[:], func=AF.Relu, bias=0.0, scale=1.0)

    ht2f = [decp.tile([128, 32], F32, tag=f"ht2f_{j}", name=f"ht2f_{j}") for j in range(8)]
    ht2 = [decp.tile([128, 32], F32R, tag=f"ht2_{j}", name=f"ht2_{j}") for j in range(8)]
    for j in range(32):
        nc.vector.transpose(ht2f[j // 4][(j % 4) * 32:(j % 4 + 1) * 32, :],
                            h2d[:, j * 32:(j + 1) * 32])
    for j in range(8):
        nc.vector.tensor_copy(out=ht2[j][:], in_=ht2f[j][:])
    rec = decp.tile([BC, 784], F32, tag="rec")
    for part, (lo, n) in enumerate([(0, 512), (512, 272)]):
        pf3 = decps.tile([BC, n], F32, tag=f"pf3_{part}")
        for j in range(8):
            nc.tensor.matmul(pf3[:], ht2[j][:], wa3[j][:, lo:lo + n],
                             start=(j == 0), stop=False)
        nc.tensor.matmul(pf3[:], onesr[:], wb3[:, lo:lo + n],
                         start=False, stop=True)
        nc.scalar.activation(out=rec[:, lo:lo + n], in_=pf3[:],
                             func=AF.Sigmoid, bias=0.0, scale=1.0)
    nc.sync.dma_start(d_rec, rec[:])
    ctx.close()


def _squash_factor(nc, pool, sq, n, tag):
    """f = sq/((1+sq)sqrt(sq)) elementwise on [P, n]."""
    s = pool.tile(list(sq.shape), F32, tag=tag + "_s")
    nc.scalar.activation(out=s[:], in_=sq[:], func=AF.Sqrt, bias=0.0, scale=1.0)
    den = pool.tile(list(sq.shape), F32, tag=tag + "_d")
    nc.vector.scalar_tensor_tensor(out=den[:], in0=sq[:], scalar=1.0,
                                   in1=s[:], op0=OP.add, op1=OP.mult)
    nc.vector.reciprocal(den[:], den[:])
    f = pool.tile(list(sq.shape), F32, tag=tag + "_f")
    nc.vector.tensor_mul(f[:], sq[:], den[:])
    return f


def _squash_scaled(nc, pool, sqr, scale, tag):
    """Given sq_raw [P,1] of raw sum o_raw, return (g, s) with
    out = o_raw * g = squash(o_raw*scale), s = sqrt(sq_scaled).
    sq_scaled = sq_raw*scale^2 ; f = sq/((1+sq)sqrt(sq)) ; g = f*scale."""
    sq = pool.tile([sqr.shape[0], 1], F32, tag=tag + "_sq")
    if scale != 1.0:
        nc.vector.tensor_scalar_mul(sq[:], sqr[:], float(scale * scale))
    else:
        nc.vector.tensor_copy(out=sq[:], in_=sqr[:])
    s = pool.tile([sqr.shape[0], 1], F32, tag=tag + "_s")
    nc.scalar.activation(out=s[:], in_=sq[:], func=AF.Sqrt, bias=0.0, scale=1.0)
    den = pool.tile([sqr.shape[0], 1], F32, tag=tag + "_d")
    nc.vector.scalar_tensor_tensor(out=den[:], in0=sq[:], scalar=1.0,
                                   in1=s[:], op0=OP.add, op1=OP.mult)
    nc.vector.reciprocal(den[:], den[:])
    f = pool.tile([sqr.shape[0], 1], F32, tag=tag + "_f")
    nc.vector.tensor_mul(f[:], sq[:], den[:])
    if scale != 1.0:
        nc.vector.tensor_scalar_mul(f[:], f[:], float(scale))
    return f, s


_NC_CACHE = None


def kernel(x, conv0_w, conv0_b, conv1_w, conv1_b, cfc_w, cfc_b, route_w,
           dec_w1, dec_b1, dec_w2, dec_b2, dec_w3, dec_b3):
    global _NC_CACHE
    if _NC_CACHE is None:
        _NC_CACHE = build_program()
    nc = _NC_CACHE

    x = np.asarray(x, np.float32)
    B = x.shape[0]
    xr = fp32r_round(x.reshape(B, 784))
    w0 = fp32r_round(np.asarray(conv0_w, np.float32).reshape(256, 81).T)
    w1 = fp32r_round(np.asarray(conv1_w, np.float32).transpose(2, 3, 1, 0)
                     .reshape(81, 2, 128, 2, 128).transpose(3, 0, 1, 2, 4).copy())
    cfc = np.concatenate(
        [np.asarray(cfc_w, np.float32),
         np.asarray(cfc_b, np.float32)[:, None, :]], axis=1
    ).transpose(1, 0, 2).reshape(257, 288).copy()
    route = np.asarray(route_w, np.float32).transpose(1, 2, 0, 3).reshape(
        36, 8, 160).copy()
    dw1 = fp32r_round(np.vstack([np.asarray(dec_w1, np.float32),
                                 np.asarray(dec_b1, np.float32)[None]]))
    dw2 = fp32r_round(np.vstack([np.asarray(dec_w2, np.float32),
                                 np.asarray(dec_b2, np.float32)[None]]))
    dw3 = fp32r_round(np.vstack([np.asarray(dec_w3, np.float32),
                                 np.asarray(dec_b3, np.float32)[None]]))
    b0 = np.ascontiguousarray(conv0_b, np.float32)
    b1 = np.ascontiguousarray(conv1_b, np.float32)

    in_maps = []
    for c in range(N_CORES):
        in_maps.append({
            "x": xr[c * BC:(c + 1) * BC], "w0": w0, "b0": b0,
            "w1": w1, "b1": b1, "cfc": cfc, "route": route,
            "dw1": dw1, "dw2": dw2, "dw3": dw3,
        })
    res = bass_utils.run_bass_kernel_spmd(
        nc, in_maps, core_ids=list(range(N_CORES)), trace=False)
    classes = np.concatenate([r["classes"] for r in res.results], axis=0)
    recon = np.concatenate([r["recon"] for r in res.results], axis=0)
    return classes, recon
